# revision 1
# baseline (speedup 1.0000x reference)
"""Trainium2 Bass kernel: windowed-LSTM local attention + linear head (LBNER).

Strategy
--------
Data-parallel over batch: B=8 sequences -> 8 NeuronCores, one sequence each.
Per core everything is laid out feature-on-partitions, L=512 on the free dim:

  xT            [768, 512]      (6 SBUF tiles of [128, 512])
  gates/P       [3072, 512]     (24 tiles of [128, 512])
  h, c          [768, 512]      (6 tiles each)

For each window size w in (3,5,7):
  P = Wih @ xT + (b_ih + b_hh)  computed ONCE (shared by all w steps; step t
  just reads P shifted by (t - w//2) columns).  Step t updates only the column
  range [s, e) that is "valid" for that offset, so out-of-range window slots
  never touch state -- this reproduces the reference's mask semantics with no
  mask tensors at all.  Step 0 has h=0 so its hidden matmul is skipped.

Recurrence per step (t >= 1):  gates_psum = WhhT.T @ h  (24 [128,512] psum
tiles, 6 K-chunks each, bf16 x bf16 -> fp32 PSUM), then per d-chunk:
  pre_g = psum + P_shift (DVE)  ->  sigmoid/tanh (ACT)  ->
  c = f*c + i*g (DVE, fp32)     ->  h = o * tanh(c) (DVE, bf16)

After the 3 windows: attn logits via elementwise mul + ones-matmul column
reduction, 3-way softmax on [1,512] rows, attention weights broadcast across
partitions with a K=1 outer-product matmul, and the residual is folded into
the head matmul: logits = lin_w @ xT + lin_w @ (sum_k attn_k * locals_k) + b.

Weights are converted to bf16 on the host; matmul accumulation is fp32 in
PSUM; the cell state c stays fp32; attention/head matmuls run plain fp32.
"""

import math
import numpy as np
import ml_dtypes

import concourse.bacc as bacc
import concourse.bass as bass
import concourse.tile as tile
from concourse import mybir
from concourse import bass_utils

B, L, D = 8, 512, 768
NL = 9
WINDOWS = (3, 5, 7)
NW = len(WINDOWS)
G4 = 4 * D          # 3072
P = 128
ND = D // P         # 6 d-chunks
NM = G4 // P        # 24 gate-chunks
N_CORES = 8

F32 = mybir.dt.float32
F32R = mybir.dt.float32r
BF16 = mybir.dt.bfloat16
AF = mybir.ActivationFunctionType


def _emit(tc, io):
    nc = tc.nc
    from contextlib import ExitStack

    with ExitStack() as ctx:
        const = ctx.enter_context(tc.tile_pool(name="const", bufs=1))
        wpool = ctx.enter_context(tc.tile_pool(name="wpool", bufs=1))
        ppool = ctx.enter_context(tc.tile_pool(name="ppool", bufs=1))
        state = ctx.enter_context(tc.tile_pool(name="state", bufs=1))
        post = ctx.enter_context(tc.tile_pool(name="post", bufs=8))
        tmp = ctx.enter_context(tc.tile_pool(name="tmp", bufs=6))
        attn = ctx.enter_context(tc.tile_pool(name="attn", bufs=7))
        logp = ctx.enter_context(tc.tile_pool(name="logp", bufs=1))
        psum = ctx.enter_context(tc.tile_pool(name="psum", bufs=8, space="PSUM"))

        # ---- constants / inputs resident in SBUF ----
        xf = []   # x.T fp32, for attention dot + residual head matmul
        xb = []   # x.T bf16, rhs of the input projections
        for dc in range(ND):
            t_f = const.tile([P, L], F32, tag=f"xf{dc}")
            nc.sync.dma_start(t_f, io["xf"].ap()[dc * P:(dc + 1) * P, :])
            xf.append(t_f)
            t_b = const.tile([P, L], BF16, tag=f"xb{dc}")
            nc.sync.dma_start(t_b, io["xb"].ap()[dc * P:(dc + 1) * P, :])
            xb.append(t_b)

        # combined LSTM bias, laid out [128, NW, NM]: partition p, window k,
        # gate-chunk m  <-  bias[k, m*128 + p]
        bias_sb = const.tile([P, NW, NM], F32, tag="bias")
        nc.sync.dma_start(
            bias_sb, io["bias"].ap().rearrange("k (m p) -> p k m", p=P)
        )

        lw = []
        for dc in range(ND):
            t = const.tile([P, NL], F32, tag=f"lw{dc}")
            nc.sync.dma_start(t, io["lwt"].ap()[dc * P:(dc + 1) * P, :])
            lw.append(t)
        lb_sb = const.tile([NL, 1], F32, tag="lb")
        nc.sync.dma_start(lb_sb, io["lb"].ap().rearrange("(c o) -> c o", o=1))

        ident_sb = const.tile([P, P], BF16, tag="ident")
        nc.sync.dma_start(ident_sb, io["ident"].ap())

        ones_col = const.tile([P, 1], F32, tag="ones_col")
        nc.vector.memset(ones_col, 1.0)
        ones_row = const.tile([1, P], F32, tag="ones_row")
        nc.vector.memset(ones_row, 1.0)

        locals_k = []   # per window: list of 6 bf16 [128, 512] tiles (final h)
        a_sb = []       # per-window attention logit rows [1, 512]
        inv_sqrt_d = 1.0 / math.sqrt(D)

        for k, w in enumerate(WINDOWS):
            hw_ = w // 2

            # ---- weights for this window (2 rotating 9.4MB slots) ----
            wih = []
            for kc in range(ND):
                t = wpool.tile([P, G4], BF16, tag=f"A{kc}")
                nc.sync.dma_start(t, io["wih"].ap()[k, kc * P:(kc + 1) * P, :])
                wih.append(t)
            whh = []
            for kc in range(ND):
                t = wpool.tile([P, G4], BF16, tag=f"B{kc}")
                nc.sync.dma_start(t, io["whh"].ap()[k, kc * P:(kc + 1) * P, :])
                whh.append(t)

            # ---- input projection: P_m = bias_m + sum_kc Wih[kc,m].T @ xT ----
            Pt = []
            for m in range(NM):
                ps = psum.tile([P, L], F32, tag="g")
                for kc in range(ND):
                    nc.tensor.matmul(
                        ps,
                        lhsT=wih[kc][:, m * P:(m + 1) * P],
                        rhs=xb[kc][:],
                        start=(kc == 0),
                        stop=(kc == ND - 1),
                    )
                pt = ppool.tile([P, L], BF16, tag=f"P{m}")
                nc.vector.tensor_scalar_add(pt, ps, bias_sb[:, k, m:m + 1])
                Pt.append(pt)

            # ---- state init ----
            c = []
            h = []
            for dc in range(ND):
                ct = state.tile([P, L], F32, tag=f"c{dc}")
                nc.gpsimd.memset(ct, 0.0)
                c.append(ct)
                ht = state.tile([P, L], BF16, tag=f"loc{k}_{dc}")
                nc.gpsimd.memset(ht, 0.0)
                h.append(ht)

            # ---- recurrence over window positions ----
            for t in range(w):
                off = t - hw_
                s = max(0, -off)
                e = min(L, L - off)
                n = e - s

                if t == 0:
                    # h == 0: gates come straight from P (bias included)
                    for dc in range(ND):
                        i_t = post.tile([P, L], BF16, tag="post")
                        nc.scalar.activation(
                            i_t[:, :n], Pt[0 + dc][:, s + off:e + off], AF.Sigmoid
                        )
                        g_t = post.tile([P, L], BF16, tag="post")
                        nc.scalar.activation(
                            g_t[:, :n], Pt[12 + dc][:, s + off:e + off], AF.Tanh
                        )
                        o_t = post.tile([P, L], BF16, tag="post")
                        nc.scalar.activation(
                            o_t[:, :n], Pt[18 + dc][:, s + off:e + off], AF.Sigmoid
                        )
                        nc.vector.tensor_mul(c[dc][:, s:e], i_t[:, :n], g_t[:, :n])
                        tc_t = post.tile([P, L], BF16, tag="post")
                        nc.scalar.activation(tc_t[:, :n], c[dc][:, s:e], AF.Tanh)
                        nc.vector.tensor_mul(h[dc][:, s:e], o_t[:, :n], tc_t[:, :n])
                    continue

                for dc in range(ND):
                    # 4 gate psum tiles for this d-chunk: i, f, g, o.
                    # P_shift (incl. bias) is folded into the accumulation
                    # with an identity matmul, so ACT reads gates from PSUM.
                    gp = []
                    for base in (0, 6, 12, 18):
                        m = base + dc
                        ps = psum.tile([P, L], F32, tag="g")
                        nc.tensor.matmul(
                            ps[:, s:e],
                            lhsT=ident_sb[:],
                            rhs=Pt[m][:, s + off:e + off],
                            start=True,
                            stop=False,
                        )
                        for kc in range(ND):
                            nc.tensor.matmul(
                                ps[:, s:e],
                                lhsT=whh[kc][:, m * P:(m + 1) * P],
                                rhs=h[kc][:, s:e],
                                start=False,
                                stop=(kc == ND - 1),
                            )
                        gp.append(ps)

                    acts = []
                    for gi, fn in enumerate(
                        (AF.Sigmoid, AF.Sigmoid, AF.Tanh, AF.Sigmoid)
                    ):
                        a = post.tile([P, L], BF16, tag="post")
                        nc.scalar.activation(a[:, :n], gp[gi][:, s:e], fn)
                        acts.append(a)
                    i_t, f_t, g_t, o_t = acts

                    t1 = tmp.tile([P, L], F32, tag="tmp")
                    nc.vector.tensor_mul(t1[:, :n], i_t[:, :n], g_t[:, :n])
                    t2 = tmp.tile([P, L], F32, tag="tmp")
                    nc.vector.tensor_mul(t2[:, :n], f_t[:, :n], c[dc][:, s:e])
                    nc.vector.tensor_add(c[dc][:, s:e], t1[:, :n], t2[:, :n])
                    tc_t = post.tile([P, L], BF16, tag="post")
                    nc.scalar.activation(tc_t[:, :n], c[dc][:, s:e], AF.Tanh)
                    nc.vector.tensor_mul(h[dc][:, s:e], o_t[:, :n], tc_t[:, :n])

            locals_k.append(h)

            # attention dot for this window, overlapped with the next window
            psd = psum.tile([1, L], F32, tag="g")
            for dc in range(ND):
                td = tmp.tile([P, L], F32, tag="tmp")
                nc.vector.tensor_mul(td, xf[dc][:], h[dc][:])
                nc.tensor.matmul(
                    psd,
                    lhsT=ones_col[:],
                    rhs=td[:],
                    start=(dc == 0),
                    stop=(dc == ND - 1),
                )
            ak = attn.tile([1, L], F32, tag=f"ak{k}", bufs=1)
            nc.scalar.activation(ak, psd, AF.Copy, scale=inv_sqrt_d)
            a_sb.append(ak)

        # ---- attention over the 3 window outputs ----
        mx1 = attn.tile([1, L], F32, tag="sm")
        nc.vector.tensor_max(mx1, a_sb[0][:], a_sb[1][:])
        mx2 = attn.tile([1, L], F32, tag="sm")
        nc.vector.tensor_max(mx2, mx1[:], a_sb[2][:])
        d_sb = []
        for k in range(NW):
            d_k = attn.tile([1, L], F32, tag="sm")
            nc.vector.tensor_sub(d_k, a_sb[k][:], mx2[:])
            d_sb.append(d_k)
        e_sb = []
        for k in range(NW):
            ek = attn.tile([1, L], F32, tag="sm")
            nc.scalar.activation(ek, d_sb[k][:], AF.Exp)
            e_sb.append(ek)
        s1 = attn.tile([1, L], F32, tag="sm")
        nc.vector.tensor_add(s1, e_sb[0][:], e_sb[1][:])
        s2 = attn.tile([1, L], F32, tag="sm")
        nc.vector.tensor_add(s2, s1[:], e_sb[2][:])
        r = attn.tile([1, L], F32, tag="sm")
        nc.vector.reciprocal(r, s2[:])

        wb = []   # attention weights broadcast to [128, 512] (PSUM)
        for k in range(NW):
            wk = attn.tile([1, L], F32, tag="sm")
            nc.vector.tensor_mul(wk, e_sb[k][:], r[:])
            pb = psum.tile([P, L], F32, tag="g")
            nc.tensor.matmul(
                pb,
                lhsT=ones_row[:],
                rhs=wk[:],
                start=True,
                stop=True,
            )
            wb.append(pb)

        # ---- head: logits = lin_w @ (x + sum_k attn_k * locals_k) + b ----
        ps_log = psum.tile([NL, L], F32, tag="g")
        for dc in range(ND):
            nc.tensor.matmul(
                ps_log,
                lhsT=lw[dc][:],
                rhs=xf[dc][:],
                start=(dc == 0),
                stop=False,
            )
        for dc in range(ND):
            lf = tmp.tile([P, L], F32, tag="tmp")
            nc.vector.tensor_mul(lf, wb[0][:], locals_k[0][dc][:])
            t3 = tmp.tile([P, L], F32, tag="tmp")
            nc.vector.tensor_mul(t3, wb[1][:], locals_k[1][dc][:])
            lf2 = tmp.tile([P, L], F32, tag="tmp")
            nc.vector.tensor_add(lf2, lf[:], t3[:])
            t4 = tmp.tile([P, L], F32, tag="tmp")
            nc.vector.tensor_mul(t4, wb[2][:], locals_k[2][dc][:])
            lf3 = tmp.tile([P, L], F32, tag="tmp")
            nc.vector.tensor_add(lf3, lf2[:], t4[:])
            nc.tensor.matmul(
                ps_log,
                lhsT=lw[dc][:],
                rhs=lf3[:],
                start=False,
                stop=(dc == ND - 1),
            )
        logits = logp.tile([NL, L], F32, tag="logits")
        nc.scalar.activation(logits, ps_log, AF.Identity, bias=lb_sb[:, 0:1])
        # store transposed: out[l, c] = logits[c, l]
        nc.sync.dma_start(io["out"].ap().rearrange("l c -> c l"), logits[:])


_NC_CACHE = {}


def _get_nc():
    if "nc" not in _NC_CACHE:
        nc = bacc.Bacc("TRN2", target_bir_lowering=False, debug=False)
        io = {
            "xf": nc.dram_tensor("xf", [D, L], F32, kind="ExternalInput"),
            "xb": nc.dram_tensor("xb", [D, L], BF16, kind="ExternalInput"),
            "wih": nc.dram_tensor("wih", [NW, D, G4], BF16, kind="ExternalInput"),
            "whh": nc.dram_tensor("whh", [NW, D, G4], BF16, kind="ExternalInput"),
            "bias": nc.dram_tensor("bias", [NW, G4], F32, kind="ExternalInput"),
            "lwt": nc.dram_tensor("lwt", [D, NL], F32, kind="ExternalInput"),
            "lb": nc.dram_tensor("lb", [NL], F32, kind="ExternalInput"),
            "ident": nc.dram_tensor("ident", [P, P], BF16, kind="ExternalInput"),
            "out": nc.dram_tensor("out", [L, NL], F32, kind="ExternalOutput"),
        }
        with tile.TileContext(nc) as tc:
            _emit(tc, io)
        nc.compile()
        _NC_CACHE["nc"] = nc
    return _NC_CACHE["nc"]


def _in_maps(sequence_output, W_ih, W_hh, b_ih, b_hh, lin_w, lin_b):
    x = np.asarray(sequence_output, np.float32)
    WihT = np.ascontiguousarray(
        np.transpose(np.asarray(W_ih, np.float32), (0, 2, 1))
    ).astype(ml_dtypes.bfloat16)
    WhhT = np.ascontiguousarray(
        np.transpose(np.asarray(W_hh, np.float32), (0, 2, 1))
    ).astype(ml_dtypes.bfloat16)
    biasc = np.asarray(b_ih, np.float32) + np.asarray(b_hh, np.float32)
    lwt = np.ascontiguousarray(np.asarray(lin_w, np.float32).T)
    lb = np.asarray(lin_b, np.float32)
    maps = []
    for b in range(B):
        xT = np.ascontiguousarray(x[b].T)
        maps.append({
            "xf": xT,
            "xb": xT.astype(ml_dtypes.bfloat16),
            "wih": WihT,
            "whh": WhhT,
            "bias": biasc,
            "lwt": lwt,
            "lb": lb,
            "ident": np.eye(P, dtype=np.float32).astype(ml_dtypes.bfloat16),
        })
    return maps


def kernel(sequence_output, W_ih, W_hh, b_ih, b_hh, lin_w, lin_b):
    nc = _get_nc()
    maps = _in_maps(sequence_output, W_ih, W_hh, b_ih, b_hh, lin_w, lin_b)
    res = bass_utils.run_bass_kernel_spmd(nc, maps, core_ids=list(range(N_CORES)))
    return np.stack([res.results[b]["out"] for b in range(B)], axis=0)


def run_traced(inputs, **kw):
    """For test.py: run with NTFF tracing, returns BassKernelResults."""
    nc = _get_nc()
    maps = _in_maps(**inputs)
    return bass_utils.run_bass_kernel_spmd(
        nc, maps, core_ids=list(range(N_CORES)), trace=True, **kw
    )



# revision 7
# speedup vs baseline: 1.0353x; 1.0353x over previous
"""Trainium2 Bass kernel: windowed-LSTM local attention + linear head (LBNER).

Strategy (v2)
-------------
Data-parallel over batch: B=8 sequences -> 8 NeuronCores, one sequence each.
Feature dim on partitions, L=512 on the free dim.

Recurrence matmuls run in fp8e4m3 with DoubleRow perf mode: each matmul
instruction contracts K=256 (two 128-chunks packed in the lhsT/rhs free
dims), so the 24 gate-chunks x 6 k-chunks of the baseline become 24 x 3
DoubleRow matmuls at half the per-row cycle cost.  The input projection
P = Wih @ x + bias uses the same trick.  Host-side numerics study:
fp8 recurrence+projection with bf16 P/activations and fp8-stored h gives
~3.6e-3 final rel err (budget 2e-2).

The h state is stored directly in fp8 as [128, 6, 512] tiles (matmul rhs
layout) and PING-PONGED between two buffers per step. This removes the
write-after-read hazard that serialized each step against the previous
step's 72 matmul reads. Correctness: valid column ranges [s_t, e_t)
shrink monotonically on both ends, so a read at step t+1 touches only
columns written at step t or never-written (zero) columns. The final
state is buffer A (t = w-1 is even) except 1-2 right-edge columns whose
last valid step was odd; those are copied from B afterwards.

PSUM: one rotating pair of [128, 4, 512] fp32 tiles (4 banks each).  Per
(step, d-chunk) the 4 banks hold gates (i, f, o, g); sigmoid runs as ONE
fused ACT op over banks 0..2, tanh(g) on bank 3.  The elementwise chain
(c = f*c + i*g, h = o*tanh(c)) is bf16 on DVE (2x mode); the h write
outputs fp8 directly.

Tail: attention dots via (x f32 * h fp8 -> bf16) + ones-matmuls, 3-way
softmax on [1,512] rows, bf16 broadcast of attention weights, head matmul
with f32r for the residual x part and bf16 for the attention part.
"""

import math
import numpy as np
import ml_dtypes

import concourse.bacc as bacc
import concourse.bass as bass
import concourse.tile as tile
from concourse import mybir
from concourse import bass_utils

B, L, D = 8, 512, 768
NL = 9
WINDOWS = (3, 5, 7)
NW = len(WINDOWS)
G4 = 4 * D          # 3072
P = 128
ND = D // P         # 6 d-chunks
NM = G4 // P        # 24 gate-chunks
NJ = ND // 2        # 3 DoubleRow k-pair chunks
N_CORES = 8

F32 = mybir.dt.float32
F32R = mybir.dt.float32r
BF16 = mybir.dt.bfloat16
FP8 = mybir.dt.float8e4
AF = mybir.ActivationFunctionType
DR = mybir.MatmulPerfMode.DoubleRow


def _emit(tc, io):
    nc = tc.nc
    from contextlib import ExitStack

    with ExitStack() as ctx:
        const = ctx.enter_context(tc.tile_pool(name="const", bufs=1))
        wpool = ctx.enter_context(tc.tile_pool(name="wpool", bufs=2))
        ppool = ctx.enter_context(tc.tile_pool(name="ppool", bufs=1))
        state = ctx.enter_context(tc.tile_pool(name="state", bufs=1))
        post = ctx.enter_context(tc.tile_pool(name="post", bufs=2))
        tmp = ctx.enter_context(tc.tile_pool(name="tmp", bufs=2))
        attn = ctx.enter_context(tc.tile_pool(name="attn", bufs=7))
        logp = ctx.enter_context(tc.tile_pool(name="logp", bufs=1))
        psum = ctx.enter_context(tc.tile_pool(name="psum", bufs=2, space="PSUM"))

        # ---- constants / inputs resident in SBUF ----
        xq_sb = const.tile([P, ND, L], FP8, tag="xq")
        nc.sync.dma_start(xq_sb, io["xq"].ap())
        xf_sb = const.tile([P, ND, L], F32R, tag="xf")
        nc.sync.dma_start(xf_sb, io["xf"].ap())

        # combined LSTM bias, [128, NW, NM]: partition p, window k, chunk m
        bias_sb = const.tile([P, NW, NM], F32, tag="bias")
        nc.sync.dma_start(
            bias_sb, io["bias"].ap().rearrange("k (m p) -> p k m", p=P)
        )

        lw_sb = const.tile([P, ND, NL], F32R, tag="lw")
        nc.sync.dma_start(
            lw_sb, io["lwt"].ap().rearrange("(c p) n -> p c n", p=P)
        )
        lwb_sb = const.tile([P, ND, NL], BF16, tag="lwb")
        nc.sync.dma_start(
            lwb_sb, io["lwtb"].ap().rearrange("(c p) n -> p c n", p=P)
        )
        lb_sb = const.tile([NL, 1], F32, tag="lb")
        nc.sync.dma_start(lb_sb, io["lb"].ap().rearrange("(c o) -> c o", o=1))

        ident_sb = const.tile([P, P], BF16, tag="ident")
        nc.sync.dma_start(ident_sb, io["ident"].ap())

        ones_col = const.tile([P, 1], BF16, tag="ones_col")
        nc.vector.memset(ones_col, 1.0)
        ones_row = const.tile([1, P], BF16, tag="ones_row")
        nc.vector.memset(ones_row, 1.0)

        hqA = []        # final per-window local states (fp8, matmul layout)
        a_sb = []       # per-window attention logit rows [1, 512] f32
        inv_sqrt_d = 1.0 / math.sqrt(D)

        for k, w in enumerate(WINDOWS):
            hw_ = w // 2

            # ---- weights for this window (double-buffered fp8) ----
            wih = wpool.tile([P, ND, G4], FP8, tag="wih")
            nc.sync.dma_start(wih, io["wihq"].ap()[k])
            whh = wpool.tile([P, ND, G4], FP8, tag="whh")
            nc.sync.dma_start(whh, io["whhq"].ap()[k])

            # ---- input projection: P_all[:, m, :] = bias_m + Wih.T @ x ----
            P_all = ppool.tile([P, NM, L], BF16, tag="P")
            for mg in range(0, NM, 4):
                ps = psum.tile([P, 4, L], F32, tag="r")
                for q in range(4):
                    m = mg + q
                    for j in range(NJ):
                        nc.tensor.matmul(
                            ps[:, q, :],
                            lhsT=wih[:, 2 * j:2 * j + 2, m * P:(m + 1) * P],
                            rhs=xq_sb[:, 2 * j:2 * j + 2, :],
                            start=(j == 0),
                            stop=(j == NJ - 1),
                            perf_mode=DR,
                        )
                for q in range(4):
                    m = mg + q
                    nc.vector.tensor_scalar_add(
                        P_all[:, m, :], ps[:, q, :], bias_sb[:, k, m:m + 1]
                    )

            # ---- state init (all zero; ping-pong h buffers) ----
            hA = state.tile([P, ND, L], FP8, tag=f"hqA{k}")
            nc.gpsimd.memset(hA, 0.0)
            hB = state.tile([P, ND, L], FP8, tag="hqB")
            nc.gpsimd.memset(hB, 0.0)
            c_all = state.tile([P, ND, L], BF16, tag="c")
            nc.gpsimd.memset(c_all, 0.0)

            # ---- t = 0: gates straight from P (h == 0), fused across dc ----
            s0, e0 = hw_, L
            n0 = e0 - s0
            si = post.tile([P, ND, L], BF16, tag="t0a", bufs=1)
            nc.scalar.activation(
                si[:, :, s0:e0], P_all[:, 0:ND, 0:n0], AF.Sigmoid
            )
            so = post.tile([P, ND, L], BF16, tag="t0b", bufs=1)
            nc.scalar.activation(
                so[:, :, s0:e0], P_all[:, 3 * ND:4 * ND, 0:n0], AF.Sigmoid
            )
            tg = post.tile([P, ND, L], BF16, tag="t0c", bufs=1)
            nc.scalar.activation(
                tg[:, :, s0:e0], P_all[:, 2 * ND:3 * ND, 0:n0], AF.Tanh
            )
            nc.vector.tensor_mul(
                c_all[:, :, s0:e0], si[:, :, s0:e0], tg[:, :, s0:e0]
            )
            tc0 = post.tile([P, ND, L], BF16, tag="t0d", bufs=1)
            nc.scalar.activation(tc0[:, :, s0:e0], c_all[:, :, s0:e0], AF.Tanh)
            nc.vector.tensor_mul(
                hA[:, :, s0:e0], so[:, :, s0:e0], tc0[:, :, s0:e0]
            )

            # ---- recurrence steps t >= 1 ----
            for t in range(1, w):
                off = t - hw_
                s = max(0, -off)
                e = min(L, L - off)
                prev, cur = (hA, hB) if t % 2 == 1 else (hB, hA)

                for dc in range(ND):
                    ms = (dc, 6 + dc, 18 + dc, 12 + dc)  # banks i, f, o, g
                    ps = psum.tile([P, 4, L], F32, tag="r")
                    for q, m in enumerate(ms):
                        nc.tensor.matmul(
                            ps[:, q, s:e],
                            lhsT=ident_sb[:],
                            rhs=P_all[:, m, s + off:e + off],
                            start=True,
                            stop=False,
                        )
                    for j in range(NJ):
                        for q, m in enumerate(ms):
                            nc.tensor.matmul(
                                ps[:, q, s:e],
                                lhsT=whh[:, 2 * j:2 * j + 2,
                                         m * P:(m + 1) * P],
                                rhs=prev[:, 2 * j:2 * j + 2, s:e],
                                start=False,
                                stop=(j == NJ - 1),
                                perf_mode=DR,
                            )

                    ifo = post.tile([P, 3, L], BF16, tag="ifo")
                    nc.scalar.activation(
                        ifo[:, :, s:e], ps[:, 0:3, s:e], AF.Sigmoid
                    )
                    g_t = post.tile([P, L], BF16, tag="g")
                    nc.scalar.activation(g_t[:, s:e], ps[:, 3, s:e], AF.Tanh)

                    t1 = tmp.tile([P, L], BF16, tag="t1")
                    nc.vector.tensor_mul(
                        t1[:, s:e], ifo[:, 0, s:e], g_t[:, s:e]
                    )
                    t2 = tmp.tile([P, L], BF16, tag="t2")
                    nc.vector.tensor_mul(
                        t2[:, s:e], ifo[:, 1, s:e], c_all[:, dc, s:e]
                    )
                    nc.vector.tensor_add(
                        c_all[:, dc, s:e], t1[:, s:e], t2[:, s:e]
                    )
                    tc_t = post.tile([P, L], BF16, tag="tc")
                    nc.scalar.activation(
                        tc_t[:, s:e], c_all[:, dc, s:e], AF.Tanh
                    )
                    nc.vector.tensor_mul(
                        cur[:, dc, s:e], ifo[:, 2, s:e], tc_t[:, s:e]
                    )

            # ---- merge: A holds final state except right-edge cols whose
            # last valid step was odd (in B) ----
            for i in range(0, hw_, 2):
                x = L - hw_ + i
                nc.vector.tensor_copy(hA[:, :, x:x + 1], hB[:, :, x:x + 1])
            hqA.append(hA)

            # ---- attention dot for this window ----
            d_all = tmp.tile([P, ND, L], BF16, tag="dot", bufs=1)
            nc.vector.tensor_mul(d_all, xf_sb[:].bitcast(F32), hA[:])
            psd = psum.tile([P, 4, L], F32, tag="r")
            for dc in range(ND):
                nc.tensor.matmul(
                    psd[0:1, 0, :],
                    lhsT=ones_col[:],
                    rhs=d_all[:, dc, :],
                    start=(dc == 0),
                    stop=(dc == ND - 1),
                )
            ak = attn.tile([1, L], F32, tag=f"ak{k}", bufs=1)
            nc.scalar.activation(ak, psd[0:1, 0, :], AF.Copy, scale=inv_sqrt_d)
            a_sb.append(ak)

        # ---- attention over the 3 window outputs ----
        mx1 = attn.tile([1, L], F32, tag="sm")
        nc.vector.tensor_max(mx1, a_sb[0][:], a_sb[1][:])
        mx2 = attn.tile([1, L], F32, tag="sm")
        nc.vector.tensor_max(mx2, mx1[:], a_sb[2][:])
        d_sb = []
        for k in range(NW):
            d_k = attn.tile([1, L], F32, tag="sm")
            nc.vector.tensor_sub(d_k, a_sb[k][:], mx2[:])
            d_sb.append(d_k)
        e_sb = []
        for k in range(NW):
            ek = attn.tile([1, L], F32, tag="sm")
            nc.scalar.activation(ek, d_sb[k][:], AF.Exp)
            e_sb.append(ek)
        s1 = attn.tile([1, L], F32, tag="sm")
        nc.vector.tensor_add(s1, e_sb[0][:], e_sb[1][:])
        s2 = attn.tile([1, L], F32, tag="sm")
        nc.vector.tensor_add(s2, s1[:], e_sb[2][:])
        r = attn.tile([1, L], F32, tag="sm")
        nc.vector.reciprocal(r, s2[:])

        # attention weights (bf16) broadcast to [128, 512] PSUM banks
        wbp = psum.tile([P, 4, L], F32, tag="r")
        for k in range(NW):
            wk = attn.tile([1, L], BF16, tag="smb")
            nc.vector.tensor_mul(wk, e_sb[k][:], r[:])
            nc.tensor.matmul(
                wbp[:, k, :],
                lhsT=ones_row[:],
                rhs=wk[:],
                start=True,
                stop=True,
            )

        # ---- head: logits = lin_w @ (x + sum_k attn_k * locals_k) + b ----
        ps_log = psum.tile([P, 4, L], F32, tag="r")
        for dc in range(ND):
            nc.tensor.matmul(
                ps_log[0:NL, 0, :],
                lhsT=lw_sb[:, dc, :],
                rhs=xf_sb[:, dc, :],
                start=(dc == 0),
                stop=False,
            )
        for dc in range(ND):
            lf = tmp.tile([P, L], BF16, tag="t1")
            nc.vector.tensor_mul(lf, wbp[:, 0, :], hqA[0][:, dc, :])
            t3 = tmp.tile([P, L], BF16, tag="t2")
            nc.vector.tensor_mul(t3, wbp[:, 1, :], hqA[1][:, dc, :])
            lf2 = tmp.tile([P, L], BF16, tag="t1")
            nc.vector.tensor_add(lf2, lf[:], t3[:])
            t4 = tmp.tile([P, L], BF16, tag="t2")
            nc.vector.tensor_mul(t4, wbp[:, 2, :], hqA[2][:, dc, :])
            lf3 = tmp.tile([P, L], BF16, tag="t1")
            nc.vector.tensor_add(lf3, lf2[:], t4[:])
            nc.tensor.matmul(
                ps_log[0:NL, 0, :],
                lhsT=lwb_sb[:, dc, :],
                rhs=lf3[:],
                start=False,
                stop=(dc == ND - 1),
            )
        logits = logp.tile([NL, L], F32, tag="logits")
        nc.scalar.activation(
            logits, ps_log[0:NL, 0, :], AF.Identity, bias=lb_sb[:, 0:1]
        )
        # store transposed: out[l, c] = logits[c, l]
        nc.sync.dma_start(io["out"].ap().rearrange("l c -> c l"), logits[:])


_NC_CACHE = {}


def _get_nc():
    if "nc" not in _NC_CACHE:
        nc = bacc.Bacc("TRN2", target_bir_lowering=False, debug=False)
        io = {
            "xq": nc.dram_tensor("xq", [P, ND, L], FP8, kind="ExternalInput"),
            "xf": nc.dram_tensor("xf", [P, ND, L], F32R, kind="ExternalInput"),
            # lwt is f32r end-to-end; f32->f32r over DMA counts as a cast
            "wihq": nc.dram_tensor(
                "wihq", [NW, P, ND, G4], FP8, kind="ExternalInput"
            ),
            "whhq": nc.dram_tensor(
                "whhq", [NW, P, ND, G4], FP8, kind="ExternalInput"
            ),
            "bias": nc.dram_tensor("bias", [NW, G4], F32, kind="ExternalInput"),
            "lwt": nc.dram_tensor("lwt", [D, NL], F32R, kind="ExternalInput"),
            "lwtb": nc.dram_tensor("lwtb", [D, NL], BF16, kind="ExternalInput"),
            "lb": nc.dram_tensor("lb", [NL], F32, kind="ExternalInput"),
            "ident": nc.dram_tensor("ident", [P, P], BF16, kind="ExternalInput"),
            "out": nc.dram_tensor("out", [L, NL], F32, kind="ExternalOutput"),
        }
        with tile.TileContext(nc) as tc:
            _emit(tc, io)
        nc.compile()
        _NC_CACHE["nc"] = nc
    return _NC_CACHE["nc"]


def _pack_w(wt):
    """[4D, D] weight -> [128, ND, 4D] fp8 (kc-chunked, partition-major)."""
    wT = np.ascontiguousarray(np.asarray(wt, np.float32).T)    # [D, 4D]
    return np.ascontiguousarray(
        wT.reshape(ND, P, G4).transpose(1, 0, 2)
    ).astype(ml_dtypes.float8_e4m3)


def _in_maps(sequence_output, W_ih, W_hh, b_ih, b_hh, lin_w, lin_b):
    x = np.asarray(sequence_output, np.float32)
    wihq = np.stack([_pack_w(W_ih[k]) for k in range(NW)])
    whhq = np.stack([_pack_w(W_hh[k]) for k in range(NW)])
    biasc = np.asarray(b_ih, np.float32) + np.asarray(b_hh, np.float32)
    lwt = np.ascontiguousarray(np.asarray(lin_w, np.float32).T)
    lwtb = lwt.astype(ml_dtypes.bfloat16)
    lb = np.asarray(lin_b, np.float32)
    ident = np.eye(P, dtype=np.float32).astype(ml_dtypes.bfloat16)
    maps = []
    for b in range(B):
        xT = np.ascontiguousarray(x[b].T)                      # [D, L]
        xp = np.ascontiguousarray(
            xT.reshape(ND, P, L).transpose(1, 0, 2)
        )                                                      # [128, ND, L]
        maps.append({
            "xq": xp.astype(ml_dtypes.float8_e4m3),
            "xf": xp,
            "wihq": wihq,
            "whhq": whhq,
            "bias": biasc,
            "lwt": lwt,
            "lwtb": lwtb,
            "lb": lb,
            "ident": ident,
        })
    return maps


def kernel(sequence_output, W_ih, W_hh, b_ih, b_hh, lin_w, lin_b):
    nc = _get_nc()
    maps = _in_maps(sequence_output, W_ih, W_hh, b_ih, b_hh, lin_w, lin_b)
    res = bass_utils.run_bass_kernel_spmd(nc, maps, core_ids=list(range(N_CORES)))
    return np.stack([res.results[b]["out"] for b in range(B)], axis=0)


def run_traced(inputs, **kw):
    """For test.py: run with NTFF tracing, returns BassKernelResults."""
    nc = _get_nc()
    maps = _in_maps(**inputs)
    return bass_utils.run_bass_kernel_spmd(
        nc, maps, core_ids=list(range(N_CORES)), trace=True, **kw
    )


# revision 18
# speedup vs baseline: 1.0661x; 1.0297x over previous
"""Trainium2 Bass kernel: windowed-LSTM local attention + linear head (LBNER).

Strategy (v2)
-------------
Data-parallel over batch: B=8 sequences -> 8 NeuronCores, one sequence each.
Feature dim on partitions, L=512 on the free dim.

Recurrence matmuls run in fp8e4m3 with DoubleRow perf mode: each matmul
instruction contracts K=256 (two 128-chunks packed in the lhsT/rhs free
dims), so the 24 gate-chunks x 6 k-chunks of the baseline become 24 x 3
DoubleRow matmuls at half the per-row cycle cost.  The input projection
P = Wih @ x + bias uses the same trick.  Host-side numerics study:
fp8 recurrence+projection with bf16 P/activations and fp8-stored h gives
~3.6e-3 final rel err (budget 2e-2).

The h state is stored directly in fp8 as [128, 6, 512] tiles (matmul rhs
layout) and PING-PONGED between two buffers per step. This removes the
write-after-read hazard that serialized each step against the previous
step's 72 matmul reads. Correctness: valid column ranges [s_t, e_t)
shrink monotonically on both ends, so a read at step t+1 touches only
columns written at step t or never-written (zero) columns. The final
state is buffer A (t = w-1 is even) except 1-2 right-edge columns whose
last valid step was odd; those are copied from B afterwards.

PSUM: one rotating pair of [128, 4, 512] fp32 tiles (4 banks each).  Per
(step, d-chunk) the 4 banks hold gates (i, f, o, g); sigmoid runs as ONE
fused ACT op over banks 0..2, tanh(g) on bank 3.  The elementwise chain
(c = f*c + i*g, h = o*tanh(c)) is bf16 on DVE (2x mode); the h write
outputs fp8 directly.

Tail: attention dots via (x f32 * h fp8 -> bf16) + ones-matmuls, 3-way
softmax on [1,512] rows, bf16 broadcast of attention weights, head matmul
with f32r for the residual x part and bf16 for the attention part.
"""

import math
import numpy as np
import ml_dtypes

import concourse.bacc as bacc
import concourse.bass as bass
import concourse.tile as tile
from concourse import mybir
from concourse import bass_utils

B, L, D = 8, 512, 768
NL = 9
WINDOWS = (3, 5, 7)
NW = len(WINDOWS)
G4 = 4 * D          # 3072
P = 128
ND = D // P         # 6 d-chunks
NM = G4 // P        # 24 gate-chunks
NJ = ND // 2        # 3 DoubleRow k-pair chunks
N_CORES = 8

F32 = mybir.dt.float32
F32R = mybir.dt.float32r
BF16 = mybir.dt.bfloat16
FP8 = mybir.dt.float8e4
AF = mybir.ActivationFunctionType
DR = mybir.MatmulPerfMode.DoubleRow


def _emit(tc, io):
    nc = tc.nc
    from contextlib import ExitStack

    with ExitStack() as ctx:
        const = ctx.enter_context(tc.tile_pool(name="const", bufs=1))
        wpool = ctx.enter_context(tc.tile_pool(name="wpool", bufs=2))
        ppool = ctx.enter_context(tc.tile_pool(name="ppool", bufs=1))
        state = ctx.enter_context(tc.tile_pool(name="state", bufs=1))
        post = ctx.enter_context(tc.tile_pool(name="post", bufs=2))
        tmp = ctx.enter_context(tc.tile_pool(name="tmp", bufs=2))
        attn = ctx.enter_context(tc.tile_pool(name="attn", bufs=7))
        logp = ctx.enter_context(tc.tile_pool(name="logp", bufs=1))
        psum = ctx.enter_context(tc.tile_pool(name="psum", bufs=4, space="PSUM"))

        # ---- constants / inputs resident in SBUF ----
        xq_sb = const.tile([P, ND, L], FP8, tag="xq")
        nc.sync.dma_start(xq_sb, io["xq"].ap())
        xf_sb = const.tile([P, ND, L], F32R, tag="xf")
        nc.sync.dma_start(xf_sb, io["xf"].ap())

        # combined LSTM bias, [128, NW, NM]: partition p, window k, chunk m
        bias_sb = const.tile([P, NW, NM], F32, tag="bias")
        nc.sync.dma_start(
            bias_sb, io["bias"].ap().rearrange("k (m p) -> p k m", p=P)
        )

        lw_sb = const.tile([P, ND, NL], F32R, tag="lw")
        nc.sync.dma_start(
            lw_sb, io["lwt"].ap().rearrange("(c p) n -> p c n", p=P)
        )
        lwb_sb = const.tile([P, ND, NL], BF16, tag="lwb")
        nc.sync.dma_start(
            lwb_sb, io["lwtb"].ap().rearrange("(c p) n -> p c n", p=P)
        )
        lb_sb = const.tile([NL, 1], F32, tag="lb")
        nc.sync.dma_start(lb_sb, io["lb"].ap().rearrange("(c o) -> c o", o=1))

        ident_sb = const.tile([P, P], BF16, tag="ident")
        nc.sync.dma_start(ident_sb, io["ident"].ap())

        ones_col = const.tile([P, 1], BF16, tag="ones_col")
        nc.vector.memset(ones_col, 1.0)
        ones_row = const.tile([1, P], BF16, tag="ones_row")
        nc.vector.memset(ones_row, 1.0)

        hqA = []        # final per-window local states (fp8, matmul layout)
        a_sb = []       # per-window attention logit rows [1, 512] f32
        inv_sqrt_d = 1.0 / math.sqrt(D)

        for k, w in enumerate(WINDOWS):
            hw_ = w // 2

            # ---- weights for this window (double-buffered fp8) ----
            wih = wpool.tile([P, ND, G4], FP8, tag="wih")
            nc.sync.dma_start(wih, io["wihq"].ap()[k])
            whh = wpool.tile([P, ND, G4], FP8, tag="whh")
            nc.sync.dma_start(whh, io["whhq"].ap()[k])

            # ---- input projection: P_all[:, m, :] = bias_m + Wih.T @ x ----
            P_all = ppool.tile([P, NM, L], BF16, tag="P")
            for mg in range(0, NM, 3):
                ps = psum.tile([P, 3, L], F32, tag="r3", bufs=2)
                for q in range(3):
                    m = mg + q
                    for j in range(NJ):
                        nc.tensor.matmul(
                            ps[:, q, :],
                            lhsT=wih[:, 2 * j:2 * j + 2, m * P:(m + 1) * P],
                            rhs=xq_sb[:, 2 * j:2 * j + 2, :],
                            start=(j == 0),
                            stop=(j == NJ - 1),
                            perf_mode=DR,
                        )
                for q in range(3):
                    m = mg + q
                    # P stored in gate order (i, f, o, g): the g and o
                    # blocks are swapped so (i, f, o) is stride-6 uniform
                    if 12 <= m < 18:
                        mloc = m + 6
                    elif 18 <= m < 24:
                        mloc = m - 6
                    else:
                        mloc = m
                    nc.vector.tensor_scalar_add(
                        P_all[:, mloc, :], ps[:, q, :], bias_sb[:, k, m:m + 1]
                    )

            # ---- state init (all zero; ping-pong h buffers) ----
            hA = state.tile([P, ND, L], FP8, tag=f"hqA{k}")
            nc.gpsimd.memset(hA, 0.0)
            hB = state.tile([P, ND, L], FP8, tag="hqB")
            nc.gpsimd.memset(hB, 0.0)
            c_all = state.tile([P, ND, L], BF16, tag="c")
            nc.gpsimd.memset(c_all, 0.0)

            # ---- t = 0: gates straight from P (h == 0), fused across dc ----
            s0, e0 = hw_, L
            n0 = e0 - s0
            si = post.tile([P, ND, L], BF16, tag="t0a", bufs=1)
            nc.scalar.activation(
                si[:, :, s0:e0], P_all[:, 0:ND, 0:n0], AF.Sigmoid
            )
            so = post.tile([P, ND, L], BF16, tag="t0b", bufs=1)
            nc.scalar.activation(
                so[:, :, s0:e0], P_all[:, 2 * ND:3 * ND, 0:n0], AF.Sigmoid
            )
            tg = post.tile([P, ND, L], BF16, tag="t0c", bufs=1)
            nc.scalar.activation(
                tg[:, :, s0:e0], P_all[:, 3 * ND:4 * ND, 0:n0], AF.Tanh
            )
            nc.vector.tensor_mul(
                c_all[:, :, s0:e0], si[:, :, s0:e0], tg[:, :, s0:e0]
            )
            tc0 = post.tile([P, ND, L], BF16, tag="t0d", bufs=1)
            nc.scalar.activation(tc0[:, :, s0:e0], c_all[:, :, s0:e0], AF.Tanh)
            nc.vector.tensor_mul(
                hA[:, :, s0:e0], so[:, :, s0:e0], tc0[:, :, s0:e0]
            )

            # ---- recurrence steps t >= 1 ----
            for t in range(1, w):
                off = t - hw_
                s = max(0, -off)
                e = min(L, L - off)
                prev, cur = (hA, hB) if t % 2 == 1 else (hB, hA)

                for dc in range(ND):
                    # (i, f, o) in a 3-bank tile (uniform m-stride 6*L) with
                    # ONE fused sigmoid; (g) in a 1-bank tile
                    psA = psum.tile([P, 3, L], F32, tag="r3", bufs=2)
                    for q in range(3):
                        nc.tensor.matmul(
                            psA[:, q, s:e],
                            lhsT=ident_sb[:],
                            rhs=P_all[:, dc + 6 * q, s + off:e + off],
                            start=True,
                            stop=False,
                            skip_group_check=True,
                        )
                    for j in range(NJ):
                        for q, m in enumerate((dc, 6 + dc, 18 + dc)):
                            nc.tensor.matmul(
                                psA[:, q, s:e],
                                lhsT=whh[:, 2 * j:2 * j + 2,
                                         m * P:(m + 1) * P],
                                rhs=prev[:, 2 * j:2 * j + 2, s:e],
                                start=False,
                                stop=(j == NJ - 1),
                                perf_mode=DR,
                                skip_group_check=True,
                            )
                    psB = psum.tile([P, 1, L], F32, tag="r1", bufs=2)
                    m = 12 + dc
                    nc.tensor.matmul(
                        psB[:, 0, s:e],
                        lhsT=ident_sb[:],
                        rhs=P_all[:, 6 + m, s + off:e + off],
                        start=True,
                        stop=False,
                        skip_group_check=True,
                    )
                    for j in range(NJ):
                        nc.tensor.matmul(
                            psB[:, 0, s:e],
                            lhsT=whh[:, 2 * j:2 * j + 2, m * P:(m + 1) * P],
                            rhs=prev[:, 2 * j:2 * j + 2, s:e],
                            start=False,
                            stop=(j == NJ - 1),
                            perf_mode=DR,
                            skip_group_check=True,
                        )
                    sifo = post.tile([P, 3, L], BF16, tag="sifo")
                    nc.scalar.activation(
                        sifo[:, :, s:e], psA[:, :, s:e], AF.Sigmoid
                    )
                    tg = post.tile([P, L], BF16, tag="tg")
                    nc.scalar.activation(tg[:, s:e], psB[:, 0, s:e], AF.Tanh)
                    t1 = tmp.tile([P, L], BF16, tag="t1")
                    nc.vector.tensor_mul(
                        t1[:, s:e], sifo[:, 0, s:e], tg[:, s:e]
                    )
                    t2 = tmp.tile([P, L], BF16, tag="t2")
                    nc.vector.tensor_mul(
                        t2[:, s:e], sifo[:, 1, s:e], c_all[:, dc, s:e]
                    )
                    nc.vector.tensor_add(
                        c_all[:, dc, s:e], t1[:, s:e], t2[:, s:e]
                    )
                    tc_t = post.tile([P, L], BF16, tag="tc")
                    nc.scalar.activation(
                        tc_t[:, s:e], c_all[:, dc, s:e], AF.Tanh
                    )
                    nc.vector.tensor_mul(
                        cur[:, dc, s:e], sifo[:, 2, s:e], tc_t[:, s:e]
                    )

            # ---- merge: A holds final state except right-edge cols whose
            # last valid step was odd (in B) ----
            for i in range(0, hw_, 2):
                x = L - hw_ + i
                nc.vector.tensor_copy(hA[:, :, x:x + 1], hB[:, :, x:x + 1])
            hqA.append(hA)

            # ---- attention dot for this window ----
            d_all = tmp.tile([P, ND, L], BF16, tag="dot", bufs=1)
            nc.vector.tensor_mul(d_all, xf_sb[:].bitcast(F32), hA[:])
            psd = psum.tile([P, 1, L], F32, tag="r1", bufs=2)
            for dc in range(ND):
                nc.tensor.matmul(
                    psd[0:1, 0, :],
                    lhsT=ones_col[:],
                    rhs=d_all[:, dc, :],
                    start=(dc == 0),
                    stop=(dc == ND - 1),
                )
            ak = attn.tile([1, L], F32, tag=f"ak{k}", bufs=1)
            nc.scalar.activation(ak, psd[0:1, 0, :], AF.Copy, scale=inv_sqrt_d)
            a_sb.append(ak)

        # ---- attention over the 3 window outputs ----
        mx1 = attn.tile([1, L], F32, tag="sm")
        nc.vector.tensor_max(mx1, a_sb[0][:], a_sb[1][:])
        mx2 = attn.tile([1, L], F32, tag="sm")
        nc.vector.tensor_max(mx2, mx1[:], a_sb[2][:])
        d_sb = []
        for k in range(NW):
            d_k = attn.tile([1, L], F32, tag="sm")
            nc.vector.tensor_sub(d_k, a_sb[k][:], mx2[:])
            d_sb.append(d_k)
        e_sb = []
        for k in range(NW):
            ek = attn.tile([1, L], F32, tag="sm")
            nc.scalar.activation(ek, d_sb[k][:], AF.Exp)
            e_sb.append(ek)
        s1 = attn.tile([1, L], F32, tag="sm")
        nc.vector.tensor_add(s1, e_sb[0][:], e_sb[1][:])
        s2 = attn.tile([1, L], F32, tag="sm")
        nc.vector.tensor_add(s2, s1[:], e_sb[2][:])
        r = attn.tile([1, L], F32, tag="sm")
        nc.vector.reciprocal(r, s2[:])

        # attention weights (bf16) broadcast to [128, 512] PSUM banks
        wbp = psum.tile([P, 3, L], F32, tag="r3", bufs=2)
        wbs = [wbp[:, 0, :], wbp[:, 1, :], wbp[:, 2, :]]
        for k in range(NW):
            wk = attn.tile([1, L], BF16, tag="smb")
            nc.vector.tensor_mul(wk, e_sb[k][:], r[:])
            nc.tensor.matmul(
                wbs[k],
                lhsT=ones_row[:],
                rhs=wk[:],
                start=True,
                stop=True,
            )

        # ---- head: logits = lin_w @ (x + sum_k attn_k * locals_k) + b ----
        ps_log = psum.tile([P, 1, L], F32, tag="r1", bufs=2)
        for dc in range(ND):
            nc.tensor.matmul(
                ps_log[0:NL, 0, :],
                lhsT=lw_sb[:, dc, :],
                rhs=xf_sb[:, dc, :],
                start=(dc == 0),
                stop=False,
            )
        for dc in range(ND):
            lf = tmp.tile([P, L], BF16, tag="t1")
            nc.vector.tensor_mul(lf, wbs[0], hqA[0][:, dc, :])
            t3 = tmp.tile([P, L], BF16, tag="t2")
            nc.vector.tensor_mul(t3, wbs[1], hqA[1][:, dc, :])
            lf2 = tmp.tile([P, L], BF16, tag="t1")
            nc.vector.tensor_add(lf2, lf[:], t3[:])
            t4 = tmp.tile([P, L], BF16, tag="t2")
            nc.vector.tensor_mul(t4, wbs[2], hqA[2][:, dc, :])
            lf3 = tmp.tile([P, L], BF16, tag="t1")
            nc.vector.tensor_add(lf3, lf2[:], t4[:])
            nc.tensor.matmul(
                ps_log[0:NL, 0, :],
                lhsT=lwb_sb[:, dc, :],
                rhs=lf3[:],
                start=False,
                stop=(dc == ND - 1),
            )
        logits = logp.tile([NL, L], F32, tag="logits")
        nc.scalar.activation(
            logits, ps_log[0:NL, 0, :], AF.Identity, bias=lb_sb[:, 0:1]
        )
        # store transposed: out[l, c] = logits[c, l]
        nc.sync.dma_start(io["out"].ap().rearrange("l c -> c l"), logits[:])


_NC_CACHE = {}


def _get_nc():
    if "nc" not in _NC_CACHE:
        nc = bacc.Bacc("TRN2", target_bir_lowering=False, debug=False)
        io = {
            "xq": nc.dram_tensor("xq", [P, ND, L], FP8, kind="ExternalInput"),
            "xf": nc.dram_tensor("xf", [P, ND, L], F32R, kind="ExternalInput"),
            # lwt is f32r end-to-end; f32->f32r over DMA counts as a cast
            "wihq": nc.dram_tensor(
                "wihq", [NW, P, ND, G4], FP8, kind="ExternalInput"
            ),
            "whhq": nc.dram_tensor(
                "whhq", [NW, P, ND, G4], FP8, kind="ExternalInput"
            ),
            "bias": nc.dram_tensor("bias", [NW, G4], F32, kind="ExternalInput"),
            "lwt": nc.dram_tensor("lwt", [D, NL], F32R, kind="ExternalInput"),
            "lwtb": nc.dram_tensor("lwtb", [D, NL], BF16, kind="ExternalInput"),
            "lb": nc.dram_tensor("lb", [NL], F32, kind="ExternalInput"),
            "ident": nc.dram_tensor("ident", [P, P], BF16, kind="ExternalInput"),
            "out": nc.dram_tensor("out", [L, NL], F32, kind="ExternalOutput"),
        }
        with tile.TileContext(nc) as tc:
            _emit(tc, io)
        nc.compile()
        _NC_CACHE["nc"] = nc
    return _NC_CACHE["nc"]


def _pack_w(wt):
    """[4D, D] weight -> [128, ND, 4D] fp8 (kc-chunked, partition-major)."""
    wT = np.ascontiguousarray(np.asarray(wt, np.float32).T)    # [D, 4D]
    return np.ascontiguousarray(
        wT.reshape(ND, P, G4).transpose(1, 0, 2)
    ).astype(ml_dtypes.float8_e4m3)


def _in_maps(sequence_output, W_ih, W_hh, b_ih, b_hh, lin_w, lin_b):
    x = np.asarray(sequence_output, np.float32)
    wihq = np.stack([_pack_w(W_ih[k]) for k in range(NW)])
    whhq = np.stack([_pack_w(W_hh[k]) for k in range(NW)])
    biasc = np.asarray(b_ih, np.float32) + np.asarray(b_hh, np.float32)
    lwt = np.ascontiguousarray(np.asarray(lin_w, np.float32).T)
    lwtb = lwt.astype(ml_dtypes.bfloat16)
    lb = np.asarray(lin_b, np.float32)
    ident = np.eye(P, dtype=np.float32).astype(ml_dtypes.bfloat16)
    maps = []
    for b in range(B):
        xT = np.ascontiguousarray(x[b].T)                      # [D, L]
        xp = np.ascontiguousarray(
            xT.reshape(ND, P, L).transpose(1, 0, 2)
        )                                                      # [128, ND, L]
        maps.append({
            "xq": xp.astype(ml_dtypes.float8_e4m3),
            "xf": xp,
            "wihq": wihq,
            "whhq": whhq,
            "bias": biasc,
            "lwt": lwt,
            "lwtb": lwtb,
            "lb": lb,
            "ident": ident,
        })
    return maps


def kernel(sequence_output, W_ih, W_hh, b_ih, b_hh, lin_w, lin_b):
    nc = _get_nc()
    maps = _in_maps(sequence_output, W_ih, W_hh, b_ih, b_hh, lin_w, lin_b)
    res = bass_utils.run_bass_kernel_spmd(nc, maps, core_ids=list(range(N_CORES)))
    return np.stack([res.results[b]["out"] for b in range(B)], axis=0)


def run_traced(inputs, **kw):
    """For test.py: run with NTFF tracing, returns BassKernelResults."""
    nc = _get_nc()
    maps = _in_maps(**inputs)
    return bass_utils.run_bass_kernel_spmd(
        nc, maps, core_ids=list(range(N_CORES)), trace=True, **kw
    )


# revision 29
# speedup vs baseline: 1.0949x; 1.0270x over previous
"""Trainium2 Bass kernel: windowed-LSTM local attention + linear head (LBNER).

Strategy (v2)
-------------
Data-parallel over batch: B=8 sequences -> 8 NeuronCores, one sequence each.
Feature dim on partitions, L=512 on the free dim.

Recurrence matmuls run in fp8e4m3 with DoubleRow perf mode: each matmul
instruction contracts K=256 (two 128-chunks packed in the lhsT/rhs free
dims), so the 24 gate-chunks x 6 k-chunks of the baseline become 24 x 3
DoubleRow matmuls at half the per-row cycle cost.  The input projection
P = Wih @ x + bias uses the same trick.  Host-side numerics study:
fp8 recurrence+projection with bf16 P/activations and fp8-stored h gives
~3.6e-3 final rel err (budget 2e-2).

The h state is stored directly in fp8 as [128, 6, 512] tiles (matmul rhs
layout) and PING-PONGED between two buffers per step. This removes the
write-after-read hazard that serialized each step against the previous
step's 72 matmul reads. Correctness: valid column ranges [s_t, e_t)
shrink monotonically on both ends, so a read at step t+1 touches only
columns written at step t or never-written (zero) columns. The final
state is buffer A (t = w-1 is even) except 1-2 right-edge columns whose
last valid step was odd; those are copied from B afterwards.

PSUM: one rotating pair of [128, 4, 512] fp32 tiles (4 banks each).  Per
(step, d-chunk) the 4 banks hold gates (i, f, o, g); sigmoid runs as ONE
fused ACT op over banks 0..2, tanh(g) on bank 3.  The elementwise chain
(c = f*c + i*g, h = o*tanh(c)) is bf16 on DVE (2x mode); the h write
outputs fp8 directly.

Tail: attention dots via (x f32 * h fp8 -> bf16) + ones-matmuls, 3-way
softmax on [1,512] rows, bf16 broadcast of attention weights, head matmul
with f32r for the residual x part and bf16 for the attention part.
"""

import math
import numpy as np
import ml_dtypes

import concourse.bacc as bacc
import concourse.bass as bass
import concourse.tile as tile
from concourse import mybir
from concourse import bass_utils

B, L, D = 8, 512, 768
NL = 9
WINDOWS = (3, 5, 7)
NW = len(WINDOWS)
G4 = 4 * D          # 3072
P = 128
ND = D // P         # 6 d-chunks
NM = G4 // P        # 24 gate-chunks
NJ = ND // 2        # 3 DoubleRow k-pair chunks
N_CORES = 8

F32 = mybir.dt.float32
F32R = mybir.dt.float32r
BF16 = mybir.dt.bfloat16
FP8 = mybir.dt.float8e4
AF = mybir.ActivationFunctionType
DR = mybir.MatmulPerfMode.DoubleRow


def _emit(tc, io):
    nc = tc.nc
    from contextlib import ExitStack

    with ExitStack() as ctx:
        const = ctx.enter_context(tc.tile_pool(name="const", bufs=1))
        wpool = ctx.enter_context(tc.tile_pool(name="wpool", bufs=2))
        ppool = ctx.enter_context(tc.tile_pool(name="ppool", bufs=1))
        state = ctx.enter_context(tc.tile_pool(name="state", bufs=1))
        post = ctx.enter_context(tc.tile_pool(name="post", bufs=2))
        tmp = ctx.enter_context(tc.tile_pool(name="tmp", bufs=2))
        attn = ctx.enter_context(tc.tile_pool(name="attn", bufs=1))
        logp = ctx.enter_context(tc.tile_pool(name="logp", bufs=1))
        psum = ctx.enter_context(tc.tile_pool(name="psum", bufs=4, space="PSUM"))

        # ---- constants / inputs resident in SBUF ----
        xq_sb = const.tile([P, ND, L], FP8, tag="xq")
        nc.sync.dma_start(xq_sb, io["xq"].ap())
        xbb_sb = const.tile([P, ND, L], BF16, tag="xbb")
        nc.sync.dma_start(xbb_sb, io["xbb"].ap())

        # combined LSTM bias, [128, NW, NM]: partition p, window k, chunk m
        bias_sb = const.tile([P, NW, NM], F32, tag="bias")
        nc.sync.dma_start(
            bias_sb, io["bias"].ap().rearrange("k (m p) -> p k m", p=P)
        )

        lwb_sb = const.tile([P, ND, NL], BF16, tag="lwb")
        nc.sync.dma_start(
            lwb_sb, io["lwtb"].ap().rearrange("(c p) n -> p c n", p=P)
        )
        lb_sb = const.tile([NL, 1], F32, tag="lb")
        nc.sync.dma_start(lb_sb, io["lb"].ap().rearrange("(c o) -> c o", o=1))

        ident_sb = const.tile([P, P], BF16, tag="ident")
        nc.sync.dma_start(ident_sb, io["ident"].ap())

        ones_col = const.tile([P, 1], BF16, tag="ones_col")
        nc.vector.memset(ones_col, 1.0)
        ones_row = const.tile([1, P], BF16, tag="ones_row")
        nc.vector.memset(ones_row, 1.0)

        hqA = {}        # final per-window local states (fp8, matmul layout)
        ak_sb = {}
        inv_sqrt_d = 1.0 / math.sqrt(D)

        st = {}         # per-window live tiles

        def load_wih(k):
            wih = wpool.tile([P, ND, G4], FP8, tag="wih", name=f"wih{k}", bufs=1)
            nc.sync.dma_start(wih, io["wihq"].ap()[k])
            st.setdefault(k, {})["wih"] = wih

        def load_whh(k):
            whh = wpool.tile([P, ND, G4], FP8, tag="whh", name=f"whh{k}")
            nc.sync.dma_start(whh, io["whhq"].ap()[k])
            st.setdefault(k, {})["whh"] = whh

        def proj(k, glo, ghi):
            """Emit projection m-groups glo..ghi (of 8 groups of 3)."""
            if "P" not in st[k]:
                st[k]["P"] = ppool.tile([P, NM, L], BF16, tag="P",
                                        name=f"P{k}", bufs=2)
            P_all = st[k]["P"]
            wih = st[k]["wih"]
            for mg in range(glo * 3, ghi * 3, 3):
                ps = psum.tile([P, 3, L], F32, tag="r3", bufs=2)
                for q in range(3):
                    m = mg + q
                    for j in range(NJ):
                        nc.tensor.matmul(
                            ps[:, q, :],
                            lhsT=wih[:, 2 * j:2 * j + 2, m * P:(m + 1) * P],
                            rhs=xq_sb[:, 2 * j:2 * j + 2, :],
                            start=(j == 0),
                            stop=(j == NJ - 1),
                            perf_mode=DR,
                        )
                for q in range(3):
                    m = mg + q
                    # P stored in gate order (i, f, o, g): the g and o
                    # blocks are swapped so (i, f, o) is stride-6 uniform
                    if 12 <= m < 18:
                        mloc = m + 6
                    elif 18 <= m < 24:
                        mloc = m - 6
                    else:
                        mloc = m
                    nc.vector.tensor_scalar_add(
                        P_all[:, mloc, :], ps[:, q, :], bias_sb[:, k, m:m + 1]
                    )

        def t0(k):
            w = WINDOWS[k]
            hw_ = w // 2
            par = k % 2
            hA = state.tile([P, ND, L], FP8, tag=f"hqA{k}", name=f"hA{k}")
            nc.gpsimd.memset(hA, 0.0)
            hB = state.tile([P, ND, L], FP8, tag=f"hqB{par}", name=f"hB{k}")
            nc.gpsimd.memset(hB, 0.0)
            c_all = state.tile([P, ND, L], BF16, tag=f"c{par}", name=f"c{k}")
            nc.gpsimd.memset(c_all, 0.0)
            st[k].update(hA=hA, hB=hB, c=c_all)
            P_all = st[k]["P"]

            s0, e0 = hw_, L
            n0 = e0 - s0
            for h0 in (0, 3):   # two 3-dc halves to bound SBUF scratch
                si = post.tile([P, 3, L], BF16, tag="t0a", bufs=1)
                nc.scalar.activation(
                    si[:, :, s0:e0], P_all[:, h0:h0 + 3, 0:n0], AF.Sigmoid
                )
                so = post.tile([P, 3, L], BF16, tag="t0b", bufs=1)
                nc.scalar.activation(
                    so[:, :, s0:e0], P_all[:, 12 + h0:15 + h0, 0:n0],
                    AF.Sigmoid
                )
                tg = post.tile([P, 3, L], BF16, tag="t0c", bufs=1)
                nc.scalar.activation(
                    tg[:, :, s0:e0], P_all[:, 18 + h0:21 + h0, 0:n0], AF.Tanh
                )
                nc.vector.tensor_mul(
                    c_all[:, h0:h0 + 3, s0:e0], si[:, :, s0:e0],
                    tg[:, :, s0:e0]
                )
                tc0 = post.tile([P, 3, L], BF16, tag="t0a", bufs=1)
                nc.scalar.activation(
                    tc0[:, :, s0:e0], c_all[:, h0:h0 + 3, s0:e0], AF.Tanh
                )
                nc.vector.tensor_mul(
                    hA[:, h0:h0 + 3, s0:e0], so[:, :, s0:e0], tc0[:, :, s0:e0]
                )

        def step_dc(k, t, dc):
            w = WINDOWS[k]
            hw_ = w // 2
            off = t - hw_
            s = max(0, -off)
            e = min(L, L - off)
            hA, hB = st[k]["hA"], st[k]["hB"]
            prev, cur = (hA, hB) if t % 2 == 1 else (hB, hA)
            whh = st[k]["whh"]
            P_all = st[k]["P"]
            c_all = st[k]["c"]

            # (i, f, o) in a 3-bank tile with ONE fused sigmoid; (g) 1-bank
            psA = psum.tile([P, 3, L], F32, tag="r3", bufs=2)
            for q in range(3):
                nc.tensor.matmul(
                    psA[:, q, s:e],
                    lhsT=ident_sb[:],
                    rhs=P_all[:, dc + 6 * q, s + off:e + off],
                    start=True,
                    stop=False,
                    skip_group_check=True,
                )
            psB = psum.tile([P, 1, L], F32, tag="r1", bufs=2)
            mg_ = 12 + dc
            nc.tensor.matmul(
                psB[:, 0, s:e],
                lhsT=ident_sb[:],
                rhs=P_all[:, 6 + mg_, s + off:e + off],
                start=True,
                stop=False,
                skip_group_check=True,
            )
            for j in range(NJ):
                for q, m in enumerate((dc, 6 + dc, 18 + dc)):
                    nc.tensor.matmul(
                        psA[:, q, s:e],
                        lhsT=whh[:, 2 * j:2 * j + 2, m * P:(m + 1) * P],
                        rhs=prev[:, 2 * j:2 * j + 2, s:e],
                        start=False,
                        stop=(j == NJ - 1),
                        perf_mode=DR,
                        skip_group_check=True,
                    )
            for j in range(NJ):
                nc.tensor.matmul(
                    psB[:, 0, s:e],
                    lhsT=whh[:, 2 * j:2 * j + 2, mg_ * P:(mg_ + 1) * P],
                    rhs=prev[:, 2 * j:2 * j + 2, s:e],
                    start=False,
                    stop=(j == NJ - 1),
                    perf_mode=DR,
                    skip_group_check=True,
                )
            sifo = post.tile([P, 3, L], BF16, tag="sifo", bufs=3)
            nc.scalar.activation(sifo[:, :, s:e], psA[:, :, s:e], AF.Sigmoid)
            tg = post.tile([P, L], BF16, tag="tg", bufs=3)
            nc.scalar.activation(tg[:, s:e], psB[:, 0, s:e], AF.Tanh)
            t1 = tmp.tile([P, L], BF16, tag="t1", bufs=2)
            nc.vector.tensor_mul(t1[:, s:e], sifo[:, 0, s:e], tg[:, s:e])
            t2 = tmp.tile([P, L], BF16, tag="t2", bufs=2)
            nc.vector.tensor_mul(
                t2[:, s:e], sifo[:, 1, s:e], c_all[:, dc, s:e]
            )
            nc.vector.tensor_add(c_all[:, dc, s:e], t1[:, s:e], t2[:, s:e])
            tc_t = post.tile([P, L], BF16, tag="tc", bufs=3)
            nc.scalar.activation(tc_t[:, s:e], c_all[:, dc, s:e], AF.Tanh)
            nc.vector.tensor_mul(
                cur[:, dc, s:e], sifo[:, 2, s:e], tc_t[:, s:e]
            )

        def finish(k):
            """Right-edge merge fixups + attention dot for window k."""
            w = WINDOWS[k]
            hw_ = w // 2
            hA, hB = st[k]["hA"], st[k]["hB"]
            for i in range(0, hw_, 2):
                x = L - hw_ + i
                nc.vector.tensor_copy(hA[:, :, x:x + 1], hB[:, :, x:x + 1])
            hqA[k] = hA
            psd = psum.tile([P, 1, L], F32, tag="r1", bufs=2)
            for h0 in (0, 3):
                d_half = tmp.tile([P, 3, L], BF16, tag="dot", bufs=1)
                nc.vector.tensor_mul(
                    d_half, xbb_sb[:, h0:h0 + 3, :], hA[:, h0:h0 + 3, :]
                )
                for q in range(3):
                    nc.tensor.matmul(
                        psd[0:1, 0, :],
                        lhsT=ones_col[:],
                        rhs=d_half[:, q, :],
                        start=(h0 == 0 and q == 0),
                        stop=(h0 == 3 and q == 2),
                    )
            ak = attn.tile([1, L], F32, tag=f"ak{k}", bufs=1, name=f"ak{k}")
            nc.scalar.activation(ak, psd[0:1, 0, :], AF.Copy, scale=inv_sqrt_d)
            ak_sb[k] = ak

        # ---- schedule: w7 (k=2) leads, w5 (k=1) trails by 2 steps, w3
        # (k=0) runs last; the trailing window's proj/t0/steps fill the
        # engine bubbles of the leading window's serial recurrence ----
        load_wih(2)
        load_whh(2)
        load_wih(1)
        load_whh(1)
        proj(2, 0, 8)
        load_wih(0)
        t0(2)
        for dc in range(ND):
            step_dc(2, 1, dc)
        proj(1, 0, 6)
        for dc in range(ND):
            step_dc(2, 2, dc)
        proj(1, 6, 8)
        t0(1)
        for dc in range(ND):
            step_dc(2, 3, dc)
            step_dc(1, 1, dc)
        for dc in range(ND):
            step_dc(2, 4, dc)
            step_dc(1, 2, dc)
        for dc in range(ND):
            step_dc(2, 5, dc)
            step_dc(1, 3, dc)
        load_whh(0)
        for dc in range(ND):
            step_dc(2, 6, dc)
            step_dc(1, 4, dc)
        proj(0, 0, 8)
        finish(2)
        t0(0)
        finish(1)
        for dc in range(ND):
            step_dc(0, 1, dc)
        for dc in range(ND):
            step_dc(0, 2, dc)
        finish(0)
        hqA = [hqA[k] for k in range(NW)]
        a_sb = [ak_sb[k] for k in range(NW)]

        # ---- attention over the 3 window outputs ----
        mx1 = attn.tile([1, L], F32, tag="sm", bufs=6)
        nc.vector.tensor_max(mx1, a_sb[0][:], a_sb[1][:])
        mx2 = attn.tile([1, L], F32, tag="sm", bufs=6)
        nc.vector.tensor_max(mx2, mx1[:], a_sb[2][:])
        d_sb = []
        for k in range(NW):
            d_k = attn.tile([1, L], F32, tag="sm", bufs=6)
            nc.vector.tensor_sub(d_k, a_sb[k][:], mx2[:])
            d_sb.append(d_k)
        e_sb = []
        for k in range(NW):
            ek = attn.tile([1, L], F32, tag="sm", bufs=6)
            nc.scalar.activation(ek, d_sb[k][:], AF.Exp)
            e_sb.append(ek)
        s1 = attn.tile([1, L], F32, tag="sm", bufs=6)
        nc.vector.tensor_add(s1, e_sb[0][:], e_sb[1][:])
        s2 = attn.tile([1, L], F32, tag="sm", bufs=6)
        nc.vector.tensor_add(s2, s1[:], e_sb[2][:])
        r = attn.tile([1, L], F32, tag="sm", bufs=6)
        nc.vector.reciprocal(r, s2[:])

        # attention weights (bf16) broadcast to [128, 512] PSUM banks
        wbp = psum.tile([P, 3, L], F32, tag="r3", bufs=2)
        wbs = [wbp[:, 0, :], wbp[:, 1, :], wbp[:, 2, :]]
        for k in range(NW):
            wk = attn.tile([1, L], BF16, tag="smb", bufs=6)
            nc.vector.tensor_mul(wk, e_sb[k][:], r[:])
            nc.tensor.matmul(
                wbs[k],
                lhsT=ones_row[:],
                rhs=wk[:],
                start=True,
                stop=True,
            )

        # ---- head: logits = lin_w @ (x + sum_k attn_k * locals_k) + b ----
        ps_log = psum.tile([P, 1, L], F32, tag="r1", bufs=2)
        for dc in range(ND):
            nc.tensor.matmul(
                ps_log[0:NL, 0, :],
                lhsT=lwb_sb[:, dc, :],
                rhs=xbb_sb[:, dc, :],
                start=(dc == 0),
                stop=False,
            )
        for dc in range(ND):
            lf = tmp.tile([P, L], BF16, tag="t1", bufs=2)
            nc.vector.tensor_mul(lf, wbs[0], hqA[0][:, dc, :])
            t3 = tmp.tile([P, L], BF16, tag="t2", bufs=2)
            nc.vector.tensor_mul(t3, wbs[1], hqA[1][:, dc, :])
            lf2 = tmp.tile([P, L], BF16, tag="t1", bufs=2)
            nc.vector.tensor_add(lf2, lf[:], t3[:])
            t4 = tmp.tile([P, L], BF16, tag="t2", bufs=2)
            nc.vector.tensor_mul(t4, wbs[2], hqA[2][:, dc, :])
            lf3 = tmp.tile([P, L], BF16, tag="t1", bufs=2)
            nc.vector.tensor_add(lf3, lf2[:], t4[:])
            nc.tensor.matmul(
                ps_log[0:NL, 0, :],
                lhsT=lwb_sb[:, dc, :],
                rhs=lf3[:],
                start=False,
                stop=(dc == ND - 1),
            )
        logits = logp.tile([NL, L], F32, tag="logits")
        nc.scalar.activation(
            logits, ps_log[0:NL, 0, :], AF.Identity, bias=lb_sb[:, 0:1]
        )
        # store transposed: out[l, c] = logits[c, l]
        nc.sync.dma_start(io["out"].ap().rearrange("l c -> c l"), logits[:])


_NC_CACHE = {}


def _get_nc():
    if "nc" not in _NC_CACHE:
        nc = bacc.Bacc("TRN2", target_bir_lowering=False, debug=False)
        io = {
            "xq": nc.dram_tensor("xq", [P, ND, L], FP8, kind="ExternalInput"),
"xbb": nc.dram_tensor("xbb", [P, ND, L], BF16, kind="ExternalInput"),
            "wihq": nc.dram_tensor(
                "wihq", [NW, P, ND, G4], FP8, kind="ExternalInput"
            ),
            "whhq": nc.dram_tensor(
                "whhq", [NW, P, ND, G4], FP8, kind="ExternalInput"
            ),
            "bias": nc.dram_tensor("bias", [NW, G4], F32, kind="ExternalInput"),
            "lwtb": nc.dram_tensor("lwtb", [D, NL], BF16, kind="ExternalInput"),
            "lb": nc.dram_tensor("lb", [NL], F32, kind="ExternalInput"),
            "ident": nc.dram_tensor("ident", [P, P], BF16, kind="ExternalInput"),
            "out": nc.dram_tensor("out", [L, NL], F32, kind="ExternalOutput"),
        }
        with tile.TileContext(nc) as tc:
            _emit(tc, io)
        nc.compile()
        _NC_CACHE["nc"] = nc
    return _NC_CACHE["nc"]


def _pack_w(wt):
    """[4D, D] weight -> [128, ND, 4D] fp8 (kc-chunked, partition-major)."""
    wT = np.ascontiguousarray(np.asarray(wt, np.float32).T)    # [D, 4D]
    return np.ascontiguousarray(
        wT.reshape(ND, P, G4).transpose(1, 0, 2)
    ).astype(ml_dtypes.float8_e4m3)


def _in_maps(sequence_output, W_ih, W_hh, b_ih, b_hh, lin_w, lin_b):
    x = np.asarray(sequence_output, np.float32)
    wihq = np.stack([_pack_w(W_ih[k]) for k in range(NW)])
    whhq = np.stack([_pack_w(W_hh[k]) for k in range(NW)])
    biasc = np.asarray(b_ih, np.float32) + np.asarray(b_hh, np.float32)
    lwtb = np.ascontiguousarray(
        np.asarray(lin_w, np.float32).T
    ).astype(ml_dtypes.bfloat16)
    lb = np.asarray(lin_b, np.float32)
    ident = np.eye(P, dtype=np.float32).astype(ml_dtypes.bfloat16)
    maps = []
    for b in range(B):
        xT = np.ascontiguousarray(x[b].T)                      # [D, L]
        xp = np.ascontiguousarray(
            xT.reshape(ND, P, L).transpose(1, 0, 2)
        )                                                      # [128, ND, L]
        maps.append({
            "xq": xp.astype(ml_dtypes.float8_e4m3),
            "xbb": xp.astype(ml_dtypes.bfloat16),
            "wihq": wihq,
            "whhq": whhq,
            "bias": biasc,
            "lwtb": lwtb,
            "lb": lb,
            "ident": ident,
        })
    return maps


def kernel(sequence_output, W_ih, W_hh, b_ih, b_hh, lin_w, lin_b):
    nc = _get_nc()
    maps = _in_maps(sequence_output, W_ih, W_hh, b_ih, b_hh, lin_w, lin_b)
    res = bass_utils.run_bass_kernel_spmd(nc, maps, core_ids=list(range(N_CORES)))
    return np.stack([res.results[b]["out"] for b in range(B)], axis=0)


def run_traced(inputs, **kw):
    """For test.py: run with NTFF tracing, returns BassKernelResults."""
    nc = _get_nc()
    maps = _in_maps(**inputs)
    return bass_utils.run_bass_kernel_spmd(
        nc, maps, core_ids=list(range(N_CORES)), trace=True, **kw
    )
